# revision 32
# baseline (speedup 1.0000x reference)
"""Trainium2 Bass kernel for nn_DecoderLayer_54855322304772.

Decoder layer with block-local attention (BLEN=256, prev+current block),
B=4, S=4096, D=1024, H=16, FF=4096.

Strategy: data-parallel over 8 cores. Tokens flattened (B*S = 16384) and
split into 8 shards of 2048 tokens (8 blocks of 256). Each core gets a
256-token halo block before its shard (zeros + full mask when the shard
starts a new sequence). No collectives.

All activations are kept feature-major ([D, T]) in SBUF so every matmul
consumes them directly (TensorE contracts over the partition axis).
LayerNorm is computed feature-major: partition-dim sums via ones-vector
matmuls, per-token stats broadcast back across partitions via K=1 f32r
matmuls. Softmax skips max-subtraction (logits are O(1); masked logits
get -1e9 and underflow to exactly 0 in exp). The softmax denominator
comes for free from a ones-column appended to V.
"""

import os
import sys

sys.path.insert(0, "/opt/trn_rl_repo")

import numpy as np
import ml_dtypes

import concourse.bass as bass
import concourse.tile as tile
from concourse import bacc, mybir
from concourse.bass_utils import run_bass_kernel_spmd
from concourse.dve_ops import RECIPROCAL_APPROX_FAST, RECIP_APPROX_FAST_CONSTS

_RC = RECIP_APPROX_FAST_CONSTS


def _recip_fast(nc, out_ap, in_ap):
    """~18-bit 1/x in a single DVE op; out may be f32r (feeds matmuls)."""
    nc.vector._custom_dve(RECIPROCAL_APPROX_FAST, out=out_ap, in0=in_ap,
                          s0=_RC["s0"], s1=_RC["s1"], imm2=_RC["imm2"])

F32 = mybir.dt.float32
F32R = mybir.dt.float32r
BF16 = mybir.dt.bfloat16
AF = mybir.ActivationFunctionType
AO = mybir.AluOpType
BFNP = ml_dtypes.bfloat16

B, S, D, H, BLEN, FF = 4, 4096, 1024, 16, 256, 4096
DH = D // H          # 64
NEG = -1e9
EPS = 1e-6
NCORES = 8
T_OWN = (B * S) // NCORES          # 2048 tokens per core
T_EXT = T_OWN + BLEN               # 2304 with halo block
NCH = D // 128                     # 8 feature chunks
NFF = FF // 128                    # 32 ff chunks
NTC = T_EXT // 128                 # 18 token chunks (for V)
QB = T_OWN // BLEN                 # 8 query blocks of 256
VW = DH + 1                        # 65: v columns per head incl. ones col

_CACHE = {}


def _ln_fm(nc, pool, psum_pool, x_chunks, tw, ones_c, ones_r, eps_ap, g_ap,
           b_ap, out_write, inv_d, name, bc_bufs=1):
    """Feature-major layernorm over D = 128*NCH (partition x chunk axis).

    x_chunks: callable c -> AP [128, tw] (bf16) input chunk c.
    out_write: callable (c, ap_f32_or_cast_src) ... we instead return per
      chunk via ACT with scale/bias into caller-provided destination:
      out_write(c, t2_ap) must emit the final op.
    """
    sums = psum_pool.tile([1, 2, tw], F32, tag=f"{name}_sums", bufs=1,
                          name=f"{name}_sums")
    sq = None
    for c in range(NCH):
        sq = pool.tile([128, tw], BF16, tag=f"{name}_sq", bufs=2,
                       name=f"{name}_sq")
        nc.scalar.activation(sq[:], x_chunks(c), AF.Square)
        nc.tensor.matmul(sums[0:1, 1, :], ones_c, sq[:],
                         start=(c == 0), stop=(c == NCH - 1))
    for c in range(NCH):
        nc.tensor.matmul(sums[0:1, 0, :], ones_c, x_chunks(c),
                         start=(c == 0), stop=(c == NCH - 1))
    stats = pool.tile([1, 2, tw], F32R, tag=f"{name}_stats", bufs=1,
                      name=f"{name}_stats")
    mu = stats[0:1, 0, :]
    rstd = stats[0:1, 1, :]
    with nc.allow_low_precision(reason="f32r stats for broadcast matmul"):
        nc.scalar.mul(mu, sums[0:1, 0, :], inv_d)
    ex2 = pool.tile([1, tw], F32, tag=f"{name}_ex2", bufs=1, name=f"{name}_ex2")
    nc.scalar.mul(ex2[:], sums[0:1, 1, :], inv_d)
    mu2 = pool.tile([1, tw], F32, tag=f"{name}_mu2", bufs=1, name=f"{name}_mu2")
    nc.vector.tensor_mul(mu2[:], mu.bitcast(F32), mu.bitcast(F32))
    var = pool.tile([1, tw], F32, tag=f"{name}_var", bufs=1, name=f"{name}_var")
    nc.vector.tensor_sub(var[:], ex2[:], mu2[:])
    sdev = pool.tile([1, tw], F32, tag=f"{name}_sdev", bufs=1,
                     name=f"{name}_sdev")
    nc.scalar.activation(sdev[:], var[:], AF.Sqrt, bias=eps_ap)
    _recip_fast(nc, rstd, sdev[:])
    bc = psum_pool.tile([128, 2, tw], F32, tag=f"{name}_bc", bufs=bc_bufs,
                        name=f"{name}_bc")
    nc.tensor.matmul(bc[:, 0, :], ones_r, mu, start=True, stop=True)
    nc.tensor.matmul(bc[:, 1, :], ones_r, rstd, start=True, stop=True)
    for c in range(NCH):
        t1 = pool.tile([128, tw], F32, tag=f"{name}_t1", bufs=2,
                       name=f"{name}_t1")
        nc.vector.tensor_sub(t1[:], x_chunks(c), bc[:, 0, :])
        t2 = pool.tile([128, tw], F32, tag=f"{name}_t2", bufs=2,
                       name=f"{name}_t2")
        nc.vector.tensor_mul(t2[:], t1[:], bc[:, 1, :])
        out_write(c, t2[:], g_ap(c), b_ap(c))


def _build():
    nc = bacc.Bacc(None, target_bir_lowering=False)

    xT = nc.dram_tensor("xT", [D, T_EXT], BF16, kind="ExternalInput")
    wq = nc.dram_tensor("wq", [D, D], BF16, kind="ExternalInput")
    wk = nc.dram_tensor("wk", [D, D], BF16, kind="ExternalInput")
    wv = nc.dram_tensor("wv", [D, D], BF16, kind="ExternalInput")
    wo = nc.dram_tensor("wo", [D, D], BF16, kind="ExternalInput")
    w1r = nc.dram_tensor("w1r", [NFF, NCH, 128, 128], BF16, kind="ExternalInput")
    w2 = nc.dram_tensor("w2", [FF, D], BF16, kind="ExternalInput")
    b1c = nc.dram_tensor("b1c", [128, NFF], F32, kind="ExternalInput")
    b2c = nc.dram_tensor("b2c", [128, NCH], F32, kind="ExternalInput")
    g1c = nc.dram_tensor("g1c", [128, NCH], F32, kind="ExternalInput")
    bb1c = nc.dram_tensor("bb1c", [128, NCH], F32, kind="ExternalInput")
    g2c = nc.dram_tensor("g2c", [128, NCH], F32, kind="ExternalInput")
    bb2c = nc.dram_tensor("bb2c", [128, NCH], F32, kind="ExternalInput")
    amask = nc.dram_tensor("amask", [128, 1024], BF16, kind="ExternalInput")
    halo01 = nc.dram_tensor("halo01", [128, 1], F32, kind="ExternalInput")
    ones_cd = nc.dram_tensor("ones_cd", [128, 1], BF16, kind="ExternalInput")
    ones_rd = nc.dram_tensor("ones_rd", [1, 128], F32R, kind="ExternalInput")
    out = nc.dram_tensor("out", [D, T_OWN], F32, kind="ExternalOutput")

    with tile.TileContext(nc) as tc:
        const = tc.alloc_tile_pool(name="const", bufs=1)
        ones_c = const.tile([128, 1], BF16)
        nc.sync.dma_start(ones_c[:], ones_cd[:])
        ones_r = const.tile([1, 128], F32R)
        nc.sync.dma_start(ones_r[:], ones_rd[:])
        eps_sb = const.tile([1, 1], F32)
        nc.vector.memset(eps_sb[:], EPS)
        b1_sb = const.tile([128, NFF], F32)
        nc.sync.dma_start(b1_sb[:], b1c[:])
        b2_sb = const.tile([128, NCH], F32)
        nc.sync.dma_start(b2_sb[:], b2c[:])
        g1_sb = const.tile([128, NCH], F32)
        nc.sync.dma_start(g1_sb[:], g1c[:])
        bb1_sb = const.tile([128, NCH], F32)
        nc.sync.dma_start(bb1_sb[:], bb1c[:])
        g2_sb = const.tile([128, NCH], F32)
        nc.sync.dma_start(g2_sb[:], g2c[:])
        bb2_sb = const.tile([128, NCH], F32)
        nc.sync.dma_start(bb2_sb[:], bb2c[:])

        # persistent x1/x2 buffer (attn+residual output, then LN1 in place)
        x12_pool = tc.alloc_tile_pool(name="x12_pool", bufs=1)
        x12_sb = x12_pool.tile([128, NCH, T_OWN], BF16)

        # resident X_ext (feature-major, bf16)
        xe_pool = tc.alloc_tile_pool(name="xe_pool", bufs=1)
        xe_sb = xe_pool.tile([128, NCH, T_EXT], BF16)
        for c in range(NCH):
            nc.sync.dma_start(xe_sb[:, c, :], xT[c * 128:(c + 1) * 128, :])

        # ---------------- Stage A: QKV projections ----------------
        qkv = tc.alloc_tile_pool(name="qkv", bufs=1)
        q_sb = qkv.tile([128, NCH, T_OWN], BF16)     # own tokens only
        k_sb = qkv.tile([128, NCH, T_EXT], BF16)
        v_sb = qkv.tile([128, NTC, H * VW], BF16)

        pa = tc.alloc_tile_pool(name="pa", bufs=1)
        psa = tc.alloc_tile_pool(name="psa", bufs=1, space="PSUM")

        # q (own tokens), k (ext tokens): stream weight slices per dout tile
        for w_dram, dst, toff, tlen in ((wq, q_sb, BLEN, T_OWN),
                                        (wk, k_sb, 0, T_EXT)):
            for dt in range(NCH):
                w_t = pa.tile([128, NCH, 128], BF16, tag="w_t", bufs=3,
                              name="w_t")
                nc.sync.dma_start(
                    w_t[:],
                    w_dram[:, dt * 128:(dt + 1) * 128].rearrange(
                        "(c p) j -> p c j", p=128))
                ntile = (tlen + 511) // 512
                for ti in range(ntile):
                    t0 = ti * 512
                    tw = min(512, tlen - t0)
                    ps = psa.tile([128, 512], F32, tag="qk_ps", bufs=2,
                                  name="qk_ps")
                    for c in range(NCH):
                        nc.tensor.matmul(
                            ps[:, 0:tw], w_t[:, c, :],
                            xe_sb[:, c, toff + t0:toff + t0 + tw],
                            start=(c == 0), stop=(c == NCH - 1))
                    if dst is q_sb:
                        nc.scalar.copy(dst[:, dt, t0:t0 + tw], ps[:, 0:tw])
                    else:
                        nc.vector.tensor_copy(dst[:, dt, t0:t0 + tw],
                                              ps[:, 0:tw])
        # v token-major with 65-stride head layout (ones col last)
        for half in range(2):
            wv_t = pa.tile([128, NCH, 512], BF16, tag="wv_t", bufs=2,
                           name="wv_t")
            nc.sync.dma_start(
                wv_t[:],
                wv[:, half * 512:(half + 1) * 512].rearrange(
                    "(c p) j -> p c j", p=128))
            for tc_g in range(NTC):
                vv = v_sb[:, tc_g, :].rearrange("p (h e) -> p h e", e=VW)
                ps = psa.tile([128, 512], F32, tag="v_ps", bufs=2,
                              name="v_ps")
                for c in range(NCH):
                    nc.tensor.matmul(
                        ps[:], xe_sb[:, c, tc_g * 128:(tc_g + 1) * 128],
                        wv_t[:, c, :],
                        start=(c == 0), stop=(c == NCH - 1))
                nc.vector.tensor_copy(
                    vv[:, half * 8:(half + 1) * 8, 0:DH],
                    ps[:].rearrange("p (h e) -> p h e", e=DH))
                if half == 0:
                    nc.vector.memset(vv[:, :, DH:VW], 1.0)

        psa.release()
        pa.release()

        # ------- Stage B: attention + Wo + residual, fused per block -------
        wo_pool = tc.alloc_tile_pool(name="wo_pool", bufs=1)
        wo_sb = wo_pool.tile([128, NCH, D], BF16)
        for c in range(NCH):
            nc.sync.dma_start(wo_sb[:, c, :], wo[c * 128:(c + 1) * 128, :])

        pb = tc.alloc_tile_pool(name="pb", bufs=1)
        psb = tc.alloc_tile_pool(name="psb", bufs=1, space="PSUM")
        amask_sb = pb.tile([128, 4, 256], BF16, tag="amask", bufs=1,
                           name="amask_sb")
        nc.sync.dma_start(amask_sb[:],
                          amask[:].rearrange("p (k q) -> p k q", k=4))
        halo_sb = pb.tile([128, 1], F32, tag="halo", bufs=1, name="halo_sb")
        nc.sync.dma_start(halo_sb[:], halo01[:])

        for qb in range(QB):
            q0 = qb * BLEN                # own-token index of query block
            k0 = qb * BLEN                # ext-token index of first key
            o_t = pb.tile([128, NCH, BLEN], BF16, tag="o_t", bufs=1,
                          name="o_t")
            for hp in range(H // 2):      # head pairs share a feature chunk
                st = psb.tile([128, 2, 4, 256], F32, tag="st", bufs=1,
                              name="st")
                for kc in range(4):       # interleave: row-groups 0/64 overlap
                    for sub in range(2):
                        p0 = sub * 64
                        nc.tensor.matmul(
                            st[:, sub, kc, :],
                            k_sb[p0:p0 + 64, hp, k0 + kc * 128:k0 + (kc + 1) * 128],
                            q_sb[p0:p0 + 64, hp, q0:q0 + BLEN],
                            start=True, stop=True)
                es = pb.tile([128, 2, 4, 256], BF16, tag="es", bufs=2,
                             name="es")
                nc.scalar.activation(es[:], st[:], AF.Exp)
                for sub in range(2):
                    nc.gpsimd.tensor_mul(es[:, sub], es[:, sub], amask_sb[:])
                if qb == 0:
                    nc.gpsimd.tensor_scalar_mul(es[:, :, 0:2, :],
                                                es[:, :, 0:2, :], halo_sb[:])
                for sub in range(2):
                    h = 2 * hp + sub
                    p0 = sub * 64
                    oa = psb.tile([VW, 256], F32, tag="oa", bufs=2, name="oa")
                    for kc in range(4):
                        nc.tensor.matmul(
                            oa[:],
                            v_sb[:, qb * 2 + kc, h * VW:(h + 1) * VW],
                            es[:, sub, kc, :],
                            start=(kc == 0), stop=(kc == 3))
                    den = pb.tile([1, 256], F32, tag="den", bufs=1,
                                  name="den")
                    if sub == 0:
                        nc.scalar.copy(den[:], oa[DH:VW, :])
                    else:
                        nc.vector.tensor_copy(den[:], oa[DH:VW, :])
                    rc = pb.tile([1, 256], F32R, tag="rc", bufs=1, name="rc")
                    _recip_fast(nc, rc[:], den[:])
                    bc = psb.tile([64, 256], F32, tag="att_bc", bufs=1,
                                  name="att_bc")
                    nc.tensor.matmul(bc[:], ones_r[0:1, 0:64], rc[:],
                                     start=True, stop=True)
                    ou = pb.tile([64, 256], BF16, tag="ou", bufs=2, name="ou")
                    if sub == 0:
                        nc.vector.tensor_copy(ou[:], oa[0:DH, :])
                    else:
                        nc.scalar.copy(ou[:], oa[0:DH, :])
                    nc.vector.tensor_mul(o_t[p0:p0 + 64, hp, :], ou[:], bc[:])
            # Wo projection + residual for this 256-token block
            for dt in range(NCH):
                y_ps = psb.tile([128, 256], F32, tag="y_ps", bufs=1,
                                name="y_ps")
                for c in range(NCH):
                    nc.tensor.matmul(y_ps[:],
                                     wo_sb[:, c, dt * 128:(dt + 1) * 128],
                                     o_t[:, c, :],
                                     start=(c == 0), stop=(c == NCH - 1))
                nc.vector.tensor_add(x12_sb[:, dt, q0:q0 + BLEN],
                                     xe_sb[:, dt, BLEN + q0:BLEN + q0 + BLEN],
                                     y_ps[:])

        psb.release()
        pb.release()
        wo_pool.release()
        qkv.release()
        xe_pool.release()

        # ---------------- Stage C: LN1 (in place on x12) ----------------
        pc = tc.alloc_tile_pool(name="pc", bufs=1)
        psc = tc.alloc_tile_pool(name="psc", bufs=1, space="PSUM")

        for tt in range(4):
            t0 = tt * 512

            def wr1(c, t2ap, g_ap, b_ap, _t0=t0):
                nc.scalar.activation(x12_sb[:, c, _t0:_t0 + 512], t2ap,
                                     AF.Identity, bias=b_ap, scale=g_ap)

            _ln_fm(nc, pc, psc,
                   (lambda c, _t0=t0: x12_sb[:, c, _t0:_t0 + 512]), 512,
                   ones_c[:], ones_r[:], eps_sb[:],
                   (lambda c: g1_sb[:, c:c + 1]),
                   (lambda c: bb1_sb[:, c:c + 1]),
                   wr1, 1.0 / D, "ln1", bc_bufs=2)

        psc.release()
        pc.release()

        # ---------------- Stage D: FFN + residual + LN2 ----------------
        w2_pool = tc.alloc_tile_pool(name="w2_pool", bufs=1)
        w2_sb = w2_pool.tile([128, NFF, D], BF16)
        for f in range(NFF):
            nc.sync.dma_start(w2_sb[:, f, :], w2[f * 128:(f + 1) * 128, :])

        pd = tc.alloc_tile_pool(name="pd", bufs=1)
        psd = tc.alloc_tile_pool(name="psd", bufs=1, space="PSUM")

        for tt in range(4):
            t0 = tt * 512
            h_sb = pd.tile([128, NFF, 512], BF16, tag="h_sb", bufs=1,
                           name="h_sb")
            for f in range(NFF):
                w1_t = pd.tile([128, NCH, 128], BF16, tag="w1_t", bufs=4,
                               name="w1_t")
                nc.sync.dma_start(
                    w1_t[:], w1r[f, :, :, :].rearrange("c p j -> p c j"))
                h_ps = psd.tile([128, 512], F32, tag="h_ps", bufs=2,
                                name="h_ps")
                for c in range(NCH):
                    nc.tensor.matmul(
                        h_ps[:], w1_t[:, c, :],
                        x12_sb[:, c, t0:t0 + 512],
                        start=(c == 0), stop=(c == NCH - 1))
                nc.scalar.activation(h_sb[:, f, :], h_ps[:], AF.Relu,
                                     bias=b1_sb[:, f:f + 1])
            x3 = pd.tile([128, NCH, 512], BF16, tag="x3", bufs=2, name="x3")
            for dt in range(NCH):
                y2_ps = psd.tile([128, 512], F32, tag="y2_ps", bufs=2,
                                 name="y2_ps")
                for f in range(NFF):
                    nc.tensor.matmul(
                        y2_ps[:], w2_sb[:, f, dt * 128:(dt + 1) * 128],
                        h_sb[:, f, :],
                        start=(f == 0), stop=(f == NFF - 1))
                yb = pd.tile([128, 512], BF16, tag="yb", bufs=2, name="yb")
                nc.scalar.activation(yb[:], y2_ps[:], AF.Identity,
                                     bias=b2_sb[:, dt:dt + 1])
                nc.vector.tensor_add(x3[:, dt, :], yb[:],
                                     x12_sb[:, dt, t0:t0 + 512])

            def wr2(c, t2ap, g_ap, b_ap, _t0=t0):
                fin = pd.tile([128, 512], F32, tag="fin", bufs=3,
                              name="fin")
                nc.scalar.activation(fin[:], t2ap, AF.Identity,
                                     bias=b_ap, scale=g_ap)
                nc.sync.dma_start(
                    out[c * 128:(c + 1) * 128, _t0:_t0 + 512], fin[:])

            _ln_fm(nc, pd, psd,
                   (lambda c, _x3=x3: _x3[:, c, :]), 512, ones_c[:],
                   ones_r[:], eps_sb[:],
                   (lambda c: g2_sb[:, c:c + 1]),
                   (lambda c: bb2_sb[:, c:c + 1]),
                   wr2, 1.0 / D, "ln2")

        psd.release()
        pd.release()
        w2_pool.release()
        x12_pool.release()
        const.release()

    nc.finalize()
    return nc


def _prep_inputs(X, Wq, Wk, Wv, Wo, g_attn, b_attn, W1, b1, W2, b2,
                 g_ffn, b_ffn):
    flat = np.ascontiguousarray(X.reshape(B * S, D))
    wq_b = (Wq * (DH ** -0.5)).astype(BFNP)
    wk_b = Wk.astype(BFNP)
    wv_b = Wv.astype(BFNP)
    wo_b = Wo.astype(BFNP)
    w1r = np.ascontiguousarray(
        W1.reshape(NCH, 128, NFF, 128).transpose(2, 0, 1, 3)).astype(BFNP)
    w2_b = W2.astype(BFNP)
    b1c = np.ascontiguousarray(b1.reshape(NFF, 128).T).astype(np.float32)
    b2c = np.ascontiguousarray(b2.reshape(NCH, 128).T).astype(np.float32)
    g1c = np.ascontiguousarray(g_attn.reshape(NCH, 128).T).astype(np.float32)
    bb1c = np.ascontiguousarray(b_attn.reshape(NCH, 128).T).astype(np.float32)
    g2c = np.ascontiguousarray(g_ffn.reshape(NCH, 128).T).astype(np.float32)
    bb2c = np.ascontiguousarray(b_ffn.reshape(NCH, 128).T).astype(np.float32)

    # multiplicative tail mask in S^T layout [key_row, (chunk, q_col)]:
    # chunk2 keeps keys r <= q i; chunk3 keeps i >= r + 128.
    r = np.arange(128)[:, None]
    i = np.arange(256)[None, :]
    tri2 = (r <= i).astype(np.float32)
    tri3 = (i >= r + 128).astype(np.float32)

    ones_cd = np.ones((128, 1), BFNP)
    ones_rd = np.ones((1, 128), np.float32)

    shared = dict(wq=wq_b, wk=wk_b, wv=wv_b, wo=wo_b, w1r=w1r, w2=w2_b,
                  b1c=b1c, b2c=b2c, g1c=g1c, bb1c=bb1c, g2c=g2c, bb2c=bb2c,
                  ones_cd=ones_cd, ones_rd=ones_rd)

    in_maps = []
    for core in range(NCORES):
        t0 = core * T_OWN
        ext = np.zeros((T_EXT, D), np.float32)
        ext[BLEN:] = flat[t0:t0 + T_OWN]
        starts_seq = (t0 % S) == 0
        if not starts_seq:
            ext[0:BLEN] = flat[t0 - BLEN:t0]
        xTc = np.ascontiguousarray(ext.T).astype(BFNP)
        am = np.empty((128, 1024), np.float32)
        am[:, 0:512] = 1.0
        am[:, 512:768] = tri2
        am[:, 768:1024] = tri3
        m = dict(shared)
        m["xT"] = xTc
        m["amask"] = am.astype(BFNP)
        m["halo01"] = np.full((128, 1), 0.0 if starts_seq else 1.0, np.float32)
        in_maps.append(m)
    return in_maps


def kernel(**inputs):
    if "nc" not in _CACHE:
        _CACHE["nc"] = _build()
    nc = _CACHE["nc"]
    in_maps = _prep_inputs(**inputs)
    trace = bool(int(os.environ.get("KERNEL_TRACE", "0")))
    if trace:
        sys.path.insert(0, os.path.dirname(os.path.abspath(__file__)))
        import types
        if "antenv.axon_hooks" not in sys.modules:
            import antenv
            from trn_agent_boot.trn_boot import _ntff_profile_via_ctypes
            hooks = types.ModuleType("antenv.axon_hooks")
            _hook = _ntff_profile_via_ctypes("/opt/axon/libaxon_pjrt.so")
            hooks.get_axon_ntff_profile_hook = lambda: _hook
            hooks.set_axon_ntff_profile_hook = lambda h: None
            sys.modules["antenv.axon_hooks"] = hooks
            antenv.axon_hooks = hooks
    res = run_bass_kernel_spmd(nc, in_maps, core_ids=list(range(NCORES)),
                               trace=trace)
    _CACHE["exec_time_ns"] = res.exec_time_ns
    out_flat = np.empty((B * S, D), np.float32)
    for core in range(NCORES):
        t0 = core * T_OWN
        out_flat[t0:t0 + T_OWN] = res.results[core]["out"].T
    return out_flat.reshape(B, S, D)


# revision 33
# speedup vs baseline: 1.1233x; 1.1233x over previous
"""Trainium2 Bass kernel for nn_DecoderLayer_54855322304772.

Decoder layer with block-local attention (BLEN=256, prev+current block),
B=4, S=4096, D=1024, H=16, FF=4096.

Strategy: data-parallel over 8 cores. Tokens flattened (B*S = 16384) and
split into 8 shards of 2048 tokens (8 blocks of 256). Each core gets a
256-token halo block before its shard (zeros + full mask when the shard
starts a new sequence). No collectives.

All activations are kept feature-major ([D, T]) in SBUF so every matmul
consumes them directly (TensorE contracts over the partition axis).
LayerNorm is computed feature-major: partition-dim sums via ones-vector
matmuls, per-token stats broadcast back across partitions via K=1 f32r
matmuls. Softmax skips max-subtraction (logits are O(1); masked logits
get -1e9 and underflow to exactly 0 in exp). The softmax denominator
comes for free from a ones-column appended to V.
"""

import os
import sys

sys.path.insert(0, "/opt/trn_rl_repo")

import numpy as np
import ml_dtypes

import concourse.bass as bass
import concourse.tile as tile
from concourse import bacc, mybir
from concourse.bass_utils import run_bass_kernel_spmd
from concourse.dve_ops import RECIPROCAL_APPROX_FAST, RECIP_APPROX_FAST_CONSTS

_RC = RECIP_APPROX_FAST_CONSTS


def _recip_fast(nc, out_ap, in_ap):
    """~18-bit 1/x in a single DVE op; out may be f32r (feeds matmuls)."""
    nc.vector._custom_dve(RECIPROCAL_APPROX_FAST, out=out_ap, in0=in_ap,
                          s0=_RC["s0"], s1=_RC["s1"], imm2=_RC["imm2"])

F32 = mybir.dt.float32
F32R = mybir.dt.float32r
BF16 = mybir.dt.bfloat16
AF = mybir.ActivationFunctionType
AO = mybir.AluOpType
BFNP = ml_dtypes.bfloat16

B, S, D, H, BLEN, FF = 4, 4096, 1024, 16, 256, 4096
DH = D // H          # 64
NEG = -1e9
EPS = 1e-6
NCORES = 8
T_OWN = (B * S) // NCORES          # 2048 tokens per core
T_EXT = T_OWN + BLEN               # 2304 with halo block
NCH = D // 128                     # 8 feature chunks
NFF = FF // 128                    # 32 ff chunks
NTC = T_EXT // 128                 # 18 token chunks (for V)
QB = T_OWN // BLEN                 # 8 query blocks of 256
VW = DH + 1                        # 65: v columns per head incl. ones col

_CACHE = {}


def _ln_fm(nc, pool, psum_pool, x_chunks, tw, ones_c, ones_r, eps_ap, g_ap,
           b_ap, out_write, inv_d, name, bc_bufs=1):
    """Feature-major layernorm over D = 128*NCH (partition x chunk axis).

    x_chunks: callable c -> AP [128, tw] (bf16) input chunk c.
    out_write: callable (c, ap_f32_or_cast_src) ... we instead return per
      chunk via ACT with scale/bias into caller-provided destination:
      out_write(c, t2_ap) must emit the final op.
    """
    sums = psum_pool.tile([1, 2, tw], F32, tag=f"{name}_sums", bufs=1,
                          name=f"{name}_sums")
    sq = None
    for c in range(NCH):
        sq = pool.tile([128, tw], BF16, tag=f"{name}_sq", bufs=2,
                       name=f"{name}_sq")
        nc.scalar.activation(sq[:], x_chunks(c), AF.Square)
        nc.tensor.matmul(sums[0:1, 1, :], ones_c, sq[:],
                         start=(c == 0), stop=(c == NCH - 1))
    for c in range(NCH):
        nc.tensor.matmul(sums[0:1, 0, :], ones_c, x_chunks(c),
                         start=(c == 0), stop=(c == NCH - 1))
    stats = pool.tile([1, 2, tw], F32R, tag=f"{name}_stats", bufs=1,
                      name=f"{name}_stats")
    mu = stats[0:1, 0, :]
    rstd = stats[0:1, 1, :]
    with nc.allow_low_precision(reason="f32r stats for broadcast matmul"):
        nc.scalar.mul(mu, sums[0:1, 0, :], inv_d)
    ex2 = pool.tile([1, tw], F32, tag=f"{name}_ex2", bufs=1, name=f"{name}_ex2")
    nc.scalar.mul(ex2[:], sums[0:1, 1, :], inv_d)
    mu2 = pool.tile([1, tw], F32, tag=f"{name}_mu2", bufs=1, name=f"{name}_mu2")
    nc.vector.tensor_mul(mu2[:], mu.bitcast(F32), mu.bitcast(F32))
    var = pool.tile([1, tw], F32, tag=f"{name}_var", bufs=1, name=f"{name}_var")
    nc.vector.tensor_sub(var[:], ex2[:], mu2[:])
    sdev = pool.tile([1, tw], F32, tag=f"{name}_sdev", bufs=1,
                     name=f"{name}_sdev")
    nc.scalar.activation(sdev[:], var[:], AF.Sqrt, bias=eps_ap)
    _recip_fast(nc, rstd, sdev[:])
    bc = psum_pool.tile([128, 2, tw], F32, tag=f"{name}_bc", bufs=bc_bufs,
                        name=f"{name}_bc")
    nc.tensor.matmul(bc[:, 0, :], ones_r, mu, start=True, stop=True)
    nc.tensor.matmul(bc[:, 1, :], ones_r, rstd, start=True, stop=True)
    for c in range(NCH):
        t1 = pool.tile([128, tw], F32, tag=f"{name}_t1", bufs=2,
                       name=f"{name}_t1")
        nc.vector.tensor_sub(t1[:], x_chunks(c), bc[:, 0, :])
        t2 = pool.tile([128, tw], F32, tag=f"{name}_t2", bufs=2,
                       name=f"{name}_t2")
        nc.vector.tensor_mul(t2[:], t1[:], bc[:, 1, :])
        out_write(c, t2[:], g_ap(c), b_ap(c))


def _build():
    nc = bacc.Bacc(None, target_bir_lowering=False)

    xT = nc.dram_tensor("xT", [D, T_EXT], BF16, kind="ExternalInput")
    wq = nc.dram_tensor("wq", [D, D], BF16, kind="ExternalInput")
    wk = nc.dram_tensor("wk", [D, D], BF16, kind="ExternalInput")
    wv = nc.dram_tensor("wv", [D, D], BF16, kind="ExternalInput")
    wo = nc.dram_tensor("wo", [D, D], BF16, kind="ExternalInput")
    w1r = nc.dram_tensor("w1r", [NFF, NCH, 128, 128], BF16, kind="ExternalInput")
    w2 = nc.dram_tensor("w2", [FF, D], BF16, kind="ExternalInput")
    b1c = nc.dram_tensor("b1c", [128, NFF], F32, kind="ExternalInput")
    b2c = nc.dram_tensor("b2c", [128, NCH], F32, kind="ExternalInput")
    g1c = nc.dram_tensor("g1c", [128, NCH], F32, kind="ExternalInput")
    bb1c = nc.dram_tensor("bb1c", [128, NCH], F32, kind="ExternalInput")
    g2c = nc.dram_tensor("g2c", [128, NCH], F32, kind="ExternalInput")
    bb2c = nc.dram_tensor("bb2c", [128, NCH], F32, kind="ExternalInput")
    amask = nc.dram_tensor("amask", [128, 1024], BF16, kind="ExternalInput")
    halo01 = nc.dram_tensor("halo01", [128, 1], F32, kind="ExternalInput")
    ones_cd = nc.dram_tensor("ones_cd", [128, 1], BF16, kind="ExternalInput")
    ones_rd = nc.dram_tensor("ones_rd", [1, 128], F32R, kind="ExternalInput")
    out = nc.dram_tensor("out", [D, T_OWN], F32, kind="ExternalOutput")

    with tile.TileContext(nc) as tc:
        const = tc.alloc_tile_pool(name="const", bufs=1)
        ones_c = const.tile([128, 1], BF16)
        nc.sync.dma_start(ones_c[:], ones_cd[:])
        ones_r = const.tile([1, 128], F32R)
        nc.sync.dma_start(ones_r[:], ones_rd[:])
        eps_sb = const.tile([1, 1], F32)
        nc.vector.memset(eps_sb[:], EPS)
        b1_sb = const.tile([128, NFF], F32)
        nc.sync.dma_start(b1_sb[:], b1c[:])
        b2_sb = const.tile([128, NCH], F32)
        nc.sync.dma_start(b2_sb[:], b2c[:])
        g1_sb = const.tile([128, NCH], F32)
        nc.sync.dma_start(g1_sb[:], g1c[:])
        bb1_sb = const.tile([128, NCH], F32)
        nc.sync.dma_start(bb1_sb[:], bb1c[:])
        g2_sb = const.tile([128, NCH], F32)
        nc.sync.dma_start(g2_sb[:], g2c[:])
        bb2_sb = const.tile([128, NCH], F32)
        nc.sync.dma_start(bb2_sb[:], bb2c[:])

        # persistent x1/x2 buffer (attn+residual output, then LN1 in place)
        x12_pool = tc.alloc_tile_pool(name="x12_pool", bufs=1)
        x12_sb = x12_pool.tile([128, NCH, T_OWN], BF16)

        # resident X_ext (feature-major, bf16)
        xe_pool = tc.alloc_tile_pool(name="xe_pool", bufs=1)
        xe_sb = xe_pool.tile([128, NCH, T_EXT], BF16)
        for c in range(NCH):
            nc.sync.dma_start(xe_sb[:, c, :], xT[c * 128:(c + 1) * 128, :])

        # ---------------- Stage A: QKV projections ----------------
        qkv = tc.alloc_tile_pool(name="qkv", bufs=1)
        q_sb = qkv.tile([128, NCH, T_OWN], BF16)     # own tokens only
        k_sb = qkv.tile([128, NCH, T_EXT], BF16)
        v_sb = qkv.tile([128, NTC, H * VW], BF16)

        pa = tc.alloc_tile_pool(name="pa", bufs=1)
        psa = tc.alloc_tile_pool(name="psa", bufs=1, space="PSUM")

        # q (own tokens), k (ext tokens): stream weight slices per dout tile
        for w_dram, dst, toff, tlen in ((wq, q_sb, BLEN, T_OWN),
                                        (wk, k_sb, 0, T_EXT)):
            for dt in range(NCH):
                w_t = pa.tile([128, NCH, 128], BF16, tag="w_t", bufs=3,
                              name="w_t")
                nc.sync.dma_start(
                    w_t[:],
                    w_dram[:, dt * 128:(dt + 1) * 128].rearrange(
                        "(c p) j -> p c j", p=128))
                ntile = (tlen + 511) // 512
                for ti in range(ntile):
                    t0 = ti * 512
                    tw = min(512, tlen - t0)
                    ps = psa.tile([128, 512], F32, tag="qk_ps", bufs=2,
                                  name="qk_ps")
                    for c in range(NCH):
                        nc.tensor.matmul(
                            ps[:, 0:tw], w_t[:, c, :],
                            xe_sb[:, c, toff + t0:toff + t0 + tw],
                            start=(c == 0), stop=(c == NCH - 1))
                    if dst is q_sb:
                        nc.scalar.copy(dst[:, dt, t0:t0 + tw], ps[:, 0:tw])
                    else:
                        nc.vector.tensor_copy(dst[:, dt, t0:t0 + tw],
                                              ps[:, 0:tw])
        # v token-major with 65-stride head layout (ones col last)
        for half in range(2):
            wv_t = pa.tile([128, NCH, 512], BF16, tag="wv_t", bufs=2,
                           name="wv_t")
            nc.sync.dma_start(
                wv_t[:],
                wv[:, half * 512:(half + 1) * 512].rearrange(
                    "(c p) j -> p c j", p=128))
            for tc_g in range(NTC):
                vv = v_sb[:, tc_g, :].rearrange("p (h e) -> p h e", e=VW)
                ps = psa.tile([128, 512], F32, tag="v_ps", bufs=2,
                              name="v_ps")
                for c in range(NCH):
                    nc.tensor.matmul(
                        ps[:], xe_sb[:, c, tc_g * 128:(tc_g + 1) * 128],
                        wv_t[:, c, :],
                        start=(c == 0), stop=(c == NCH - 1))
                nc.vector.tensor_copy(
                    vv[:, half * 8:(half + 1) * 8, 0:DH],
                    ps[:].rearrange("p (h e) -> p h e", e=DH))
                if half == 0:
                    nc.vector.memset(vv[:, :, DH:VW], 1.0)

        psa.release()
        pa.release()

        # ------- Stage B: attention + Wo + residual, fused per block -------
        wo_pool = tc.alloc_tile_pool(name="wo_pool", bufs=1)
        wo_sb = wo_pool.tile([128, NCH, D], BF16)
        for c in range(NCH):
            nc.sync.dma_start(wo_sb[:, c, :], wo[c * 128:(c + 1) * 128, :])

        pb = tc.alloc_tile_pool(name="pb", bufs=1)
        psb = tc.alloc_tile_pool(name="psb", bufs=1, space="PSUM")
        amask_sb = pb.tile([128, 4, 256], BF16, tag="amask", bufs=1,
                           name="amask_sb")
        nc.sync.dma_start(amask_sb[:],
                          amask[:].rearrange("p (k q) -> p k q", k=4))
        halo_sb = pb.tile([128, 1], F32, tag="halo", bufs=1, name="halo_sb")
        nc.sync.dma_start(halo_sb[:], halo01[:])

        for qb in range(QB):
            q0 = qb * BLEN                # own-token index of query block
            k0 = qb * BLEN                # ext-token index of first key
            o_t = pb.tile([128, NCH, BLEN], BF16, tag="o_t", bufs=1,
                          name="o_t")
            for hp in range(H // 2):      # head pairs share a feature chunk
                st = psb.tile([128, 2, 4, 256], F32, tag="st", bufs=1,
                              name="st")
                for kc in range(4):       # interleave: row-groups 0/64 overlap
                    for sub in range(2):
                        p0 = sub * 64
                        nc.tensor.matmul(
                            st[:, sub, kc, :],
                            k_sb[p0:p0 + 64, hp, k0 + kc * 128:k0 + (kc + 1) * 128],
                            q_sb[p0:p0 + 64, hp, q0:q0 + BLEN],
                            start=True, stop=True)
                es = pb.tile([128, 2, 4, 256], BF16, tag="es", bufs=2,
                             name="es")
                nc.scalar.activation(es[:], st[:], AF.Exp)
                for sub in range(2):
                    nc.vector.tensor_mul(es[:, sub], es[:, sub], amask_sb[:])
                if qb == 0:
                    nc.vector.tensor_scalar_mul(es[:, :, 0:2, :],
                                                es[:, :, 0:2, :], halo_sb[:])
                for sub in range(2):
                    h = 2 * hp + sub
                    p0 = sub * 64
                    oa = psb.tile([VW, 256], F32, tag="oa", bufs=2, name="oa")
                    for kc in range(4):
                        nc.tensor.matmul(
                            oa[:],
                            v_sb[:, qb * 2 + kc, h * VW:(h + 1) * VW],
                            es[:, sub, kc, :],
                            start=(kc == 0), stop=(kc == 3))
                    den = pb.tile([1, 256], F32, tag="den", bufs=1,
                                  name="den")
                    if sub == 0:
                        nc.scalar.copy(den[:], oa[DH:VW, :])
                    else:
                        nc.vector.tensor_copy(den[:], oa[DH:VW, :])
                    rc = pb.tile([1, 256], F32R, tag="rc", bufs=1, name="rc")
                    _recip_fast(nc, rc[:], den[:])
                    bc = psb.tile([64, 256], F32, tag="att_bc", bufs=1,
                                  name="att_bc")
                    nc.tensor.matmul(bc[:], ones_r[0:1, 0:64], rc[:],
                                     start=True, stop=True)
                    ou = pb.tile([64, 256], BF16, tag="ou", bufs=2, name="ou")
                    if sub == 0:
                        nc.vector.tensor_copy(ou[:], oa[0:DH, :])
                    else:
                        nc.scalar.copy(ou[:], oa[0:DH, :])
                    nc.vector.tensor_mul(o_t[p0:p0 + 64, hp, :], ou[:], bc[:])
            # Wo projection + residual for this 256-token block
            for dt in range(NCH):
                y_ps = psb.tile([128, 256], F32, tag="y_ps", bufs=1,
                                name="y_ps")
                for c in range(NCH):
                    nc.tensor.matmul(y_ps[:],
                                     wo_sb[:, c, dt * 128:(dt + 1) * 128],
                                     o_t[:, c, :],
                                     start=(c == 0), stop=(c == NCH - 1))
                nc.vector.tensor_add(x12_sb[:, dt, q0:q0 + BLEN],
                                     xe_sb[:, dt, BLEN + q0:BLEN + q0 + BLEN],
                                     y_ps[:])

        psb.release()
        pb.release()
        wo_pool.release()
        qkv.release()
        xe_pool.release()

        # ---------------- Stage C: LN1 (in place on x12) ----------------
        w2_pool = tc.alloc_tile_pool(name="w2_pool", bufs=1)
        w2_sb = w2_pool.tile([128, NFF, D], BF16)
        for f in range(NFF):
            nc.sync.dma_start(w2_sb[:, f, :], w2[f * 128:(f + 1) * 128, :])

        pc = tc.alloc_tile_pool(name="pc", bufs=1)
        psc = tc.alloc_tile_pool(name="psc", bufs=1, space="PSUM")

        for tt in range(4):
            t0 = tt * 512

            def wr1(c, t2ap, g_ap, b_ap, _t0=t0):
                nc.scalar.activation(x12_sb[:, c, _t0:_t0 + 512], t2ap,
                                     AF.Identity, bias=b_ap, scale=g_ap)

            _ln_fm(nc, pc, psc,
                   (lambda c, _t0=t0: x12_sb[:, c, _t0:_t0 + 512]), 512,
                   ones_c[:], ones_r[:], eps_sb[:],
                   (lambda c: g1_sb[:, c:c + 1]),
                   (lambda c: bb1_sb[:, c:c + 1]),
                   wr1, 1.0 / D, "ln1", bc_bufs=2)

        psc.release()
        pc.release()

        # ---------------- Stage D: FFN + residual + LN2 ----------------
        pd = tc.alloc_tile_pool(name="pd", bufs=1)
        psd = tc.alloc_tile_pool(name="psd", bufs=1, space="PSUM")

        for tt in range(4):
            t0 = tt * 512
            h_sb = pd.tile([128, NFF, 512], BF16, tag="h_sb", bufs=1,
                           name="h_sb")
            for f in range(NFF):
                w1_t = pd.tile([128, NCH, 128], BF16, tag="w1_t", bufs=4,
                               name="w1_t")
                nc.sync.dma_start(
                    w1_t[:], w1r[f, :, :, :].rearrange("c p j -> p c j"))
                h_ps = psd.tile([128, 512], F32, tag="h_ps", bufs=2,
                                name="h_ps")
                for c in range(NCH):
                    nc.tensor.matmul(
                        h_ps[:], w1_t[:, c, :],
                        x12_sb[:, c, t0:t0 + 512],
                        start=(c == 0), stop=(c == NCH - 1))
                nc.scalar.activation(h_sb[:, f, :], h_ps[:], AF.Relu,
                                     bias=b1_sb[:, f:f + 1])
            x3 = pd.tile([128, NCH, 512], BF16, tag="x3", bufs=2, name="x3")
            for dt in range(NCH):
                y2_ps = psd.tile([128, 512], F32, tag="y2_ps", bufs=2,
                                 name="y2_ps")
                for f in range(NFF):
                    nc.tensor.matmul(
                        y2_ps[:], w2_sb[:, f, dt * 128:(dt + 1) * 128],
                        h_sb[:, f, :],
                        start=(f == 0), stop=(f == NFF - 1))
                yb = pd.tile([128, 512], BF16, tag="yb", bufs=2, name="yb")
                nc.scalar.activation(yb[:], y2_ps[:], AF.Identity,
                                     bias=b2_sb[:, dt:dt + 1])
                nc.vector.tensor_add(x3[:, dt, :], yb[:],
                                     x12_sb[:, dt, t0:t0 + 512])

            def wr2(c, t2ap, g_ap, b_ap, _t0=t0):
                fin = pd.tile([128, 512], F32, tag="fin", bufs=3,
                              name="fin")
                nc.scalar.activation(fin[:], t2ap, AF.Identity,
                                     bias=b_ap, scale=g_ap)
                nc.sync.dma_start(
                    out[c * 128:(c + 1) * 128, _t0:_t0 + 512], fin[:])

            _ln_fm(nc, pd, psd,
                   (lambda c, _x3=x3: _x3[:, c, :]), 512, ones_c[:],
                   ones_r[:], eps_sb[:],
                   (lambda c: g2_sb[:, c:c + 1]),
                   (lambda c: bb2_sb[:, c:c + 1]),
                   wr2, 1.0 / D, "ln2")

        psd.release()
        pd.release()
        w2_pool.release()
        x12_pool.release()
        const.release()

    nc.finalize()
    return nc


def _prep_inputs(X, Wq, Wk, Wv, Wo, g_attn, b_attn, W1, b1, W2, b2,
                 g_ffn, b_ffn):
    flat = np.ascontiguousarray(X.reshape(B * S, D))
    wq_b = (Wq * (DH ** -0.5)).astype(BFNP)
    wk_b = Wk.astype(BFNP)
    wv_b = Wv.astype(BFNP)
    wo_b = Wo.astype(BFNP)
    w1r = np.ascontiguousarray(
        W1.reshape(NCH, 128, NFF, 128).transpose(2, 0, 1, 3)).astype(BFNP)
    w2_b = W2.astype(BFNP)
    b1c = np.ascontiguousarray(b1.reshape(NFF, 128).T).astype(np.float32)
    b2c = np.ascontiguousarray(b2.reshape(NCH, 128).T).astype(np.float32)
    g1c = np.ascontiguousarray(g_attn.reshape(NCH, 128).T).astype(np.float32)
    bb1c = np.ascontiguousarray(b_attn.reshape(NCH, 128).T).astype(np.float32)
    g2c = np.ascontiguousarray(g_ffn.reshape(NCH, 128).T).astype(np.float32)
    bb2c = np.ascontiguousarray(b_ffn.reshape(NCH, 128).T).astype(np.float32)

    # multiplicative tail mask in S^T layout [key_row, (chunk, q_col)]:
    # chunk2 keeps keys r <= q i; chunk3 keeps i >= r + 128.
    r = np.arange(128)[:, None]
    i = np.arange(256)[None, :]
    tri2 = (r <= i).astype(np.float32)
    tri3 = (i >= r + 128).astype(np.float32)

    ones_cd = np.ones((128, 1), BFNP)
    ones_rd = np.ones((1, 128), np.float32)

    shared = dict(wq=wq_b, wk=wk_b, wv=wv_b, wo=wo_b, w1r=w1r, w2=w2_b,
                  b1c=b1c, b2c=b2c, g1c=g1c, bb1c=bb1c, g2c=g2c, bb2c=bb2c,
                  ones_cd=ones_cd, ones_rd=ones_rd)

    in_maps = []
    for core in range(NCORES):
        t0 = core * T_OWN
        ext = np.zeros((T_EXT, D), np.float32)
        ext[BLEN:] = flat[t0:t0 + T_OWN]
        starts_seq = (t0 % S) == 0
        if not starts_seq:
            ext[0:BLEN] = flat[t0 - BLEN:t0]
        xTc = np.ascontiguousarray(ext.T).astype(BFNP)
        am = np.empty((128, 1024), np.float32)
        am[:, 0:512] = 1.0
        am[:, 512:768] = tri2
        am[:, 768:1024] = tri3
        m = dict(shared)
        m["xT"] = xTc
        m["amask"] = am.astype(BFNP)
        m["halo01"] = np.full((128, 1), 0.0 if starts_seq else 1.0, np.float32)
        in_maps.append(m)
    return in_maps


def kernel(**inputs):
    if "nc" not in _CACHE:
        _CACHE["nc"] = _build()
    nc = _CACHE["nc"]
    in_maps = _prep_inputs(**inputs)
    trace = bool(int(os.environ.get("KERNEL_TRACE", "0")))
    if trace:
        sys.path.insert(0, os.path.dirname(os.path.abspath(__file__)))
        import types
        if "antenv.axon_hooks" not in sys.modules:
            import antenv
            from trn_agent_boot.trn_boot import _ntff_profile_via_ctypes
            hooks = types.ModuleType("antenv.axon_hooks")
            _hook = _ntff_profile_via_ctypes("/opt/axon/libaxon_pjrt.so")
            hooks.get_axon_ntff_profile_hook = lambda: _hook
            hooks.set_axon_ntff_profile_hook = lambda h: None
            sys.modules["antenv.axon_hooks"] = hooks
            antenv.axon_hooks = hooks
    res = run_bass_kernel_spmd(nc, in_maps, core_ids=list(range(NCORES)),
                               trace=trace)
    _CACHE["exec_time_ns"] = res.exec_time_ns
    out_flat = np.empty((B * S, D), np.float32)
    for core in range(NCORES):
        t0 = core * T_OWN
        out_flat[t0:t0 + T_OWN] = res.results[core]["out"].T
    return out_flat.reshape(B, S, D)


# revision 34
# speedup vs baseline: 1.1300x; 1.0060x over previous
"""Trainium2 Bass kernel for nn_DecoderLayer_54855322304772.

Decoder layer with block-local attention (BLEN=256, prev+current block),
B=4, S=4096, D=1024, H=16, FF=4096.

Strategy: data-parallel over 8 cores. Tokens flattened (B*S = 16384) and
split into 8 shards of 2048 tokens (8 blocks of 256). Each core gets a
256-token halo block before its shard (zeros + full mask when the shard
starts a new sequence). No collectives.

All activations are kept feature-major ([D, T]) in SBUF so every matmul
consumes them directly (TensorE contracts over the partition axis).
LayerNorm is computed feature-major: partition-dim sums via ones-vector
matmuls, per-token stats broadcast back across partitions via K=1 f32r
matmuls. Softmax skips max-subtraction (logits are O(1); masked logits
get -1e9 and underflow to exactly 0 in exp). The softmax denominator
comes for free from a ones-column appended to V.
"""

import os
import sys

sys.path.insert(0, "/opt/trn_rl_repo")

import numpy as np
import ml_dtypes

import concourse.bass as bass
import concourse.tile as tile
from concourse import bacc, mybir
from concourse.bass_utils import run_bass_kernel_spmd
from concourse.dve_ops import RECIPROCAL_APPROX_FAST, RECIP_APPROX_FAST_CONSTS

_RC = RECIP_APPROX_FAST_CONSTS


def _recip_fast(nc, out_ap, in_ap):
    """~18-bit 1/x in a single DVE op; out may be f32r (feeds matmuls)."""
    nc.vector._custom_dve(RECIPROCAL_APPROX_FAST, out=out_ap, in0=in_ap,
                          s0=_RC["s0"], s1=_RC["s1"], imm2=_RC["imm2"])

F32 = mybir.dt.float32
F32R = mybir.dt.float32r
BF16 = mybir.dt.bfloat16
AF = mybir.ActivationFunctionType
AO = mybir.AluOpType
BFNP = ml_dtypes.bfloat16

B, S, D, H, BLEN, FF = 4, 4096, 1024, 16, 256, 4096
DH = D // H          # 64
NEG = -1e9
EPS = 1e-6
NCORES = 8
T_OWN = (B * S) // NCORES          # 2048 tokens per core
T_EXT = T_OWN + BLEN               # 2304 with halo block
NCH = D // 128                     # 8 feature chunks
NFF = FF // 128                    # 32 ff chunks
NTC = T_EXT // 128                 # 18 token chunks (for V)
QB = T_OWN // BLEN                 # 8 query blocks of 256
VW = DH + 1                        # 65: v columns per head incl. ones col

_CACHE = {}


def _ln_fm(nc, pool, psum_pool, x_chunks, tw, ones_c, ones_r, eps_ap, g_ap,
           b_ap, out_write, inv_d, name, bc_bufs=1):
    """Feature-major layernorm over D = 128*NCH (partition x chunk axis).

    x_chunks: callable c -> AP [128, tw] (bf16) input chunk c.
    out_write: callable (c, ap_f32_or_cast_src) ... we instead return per
      chunk via ACT with scale/bias into caller-provided destination:
      out_write(c, t2_ap) must emit the final op.
    """
    sums = psum_pool.tile([1, 2, tw], F32, tag=f"{name}_sums", bufs=1,
                          name=f"{name}_sums")
    sq = None
    for c in range(NCH):
        sq = pool.tile([128, tw], BF16, tag=f"{name}_sq", bufs=2,
                       name=f"{name}_sq")
        nc.scalar.activation(sq[:], x_chunks(c), AF.Square)
        nc.tensor.matmul(sums[0:1, 1, :], ones_c, sq[:],
                         start=(c == 0), stop=(c == NCH - 1))
    for c in range(NCH):
        nc.tensor.matmul(sums[0:1, 0, :], ones_c, x_chunks(c),
                         start=(c == 0), stop=(c == NCH - 1))
    stats = pool.tile([1, 2, tw], F32R, tag=f"{name}_stats", bufs=1,
                      name=f"{name}_stats")
    mu = stats[0:1, 0, :]
    rstd = stats[0:1, 1, :]
    with nc.allow_low_precision(reason="f32r stats for broadcast matmul"):
        nc.scalar.mul(mu, sums[0:1, 0, :], inv_d)
    ex2 = pool.tile([1, tw], F32, tag=f"{name}_ex2", bufs=1, name=f"{name}_ex2")
    nc.scalar.mul(ex2[:], sums[0:1, 1, :], inv_d)
    mu2 = pool.tile([1, tw], F32, tag=f"{name}_mu2", bufs=1, name=f"{name}_mu2")
    nc.vector.tensor_mul(mu2[:], mu.bitcast(F32), mu.bitcast(F32))
    var = pool.tile([1, tw], F32, tag=f"{name}_var", bufs=1, name=f"{name}_var")
    nc.vector.tensor_sub(var[:], ex2[:], mu2[:])
    sdev = pool.tile([1, tw], F32, tag=f"{name}_sdev", bufs=1,
                     name=f"{name}_sdev")
    nc.scalar.activation(sdev[:], var[:], AF.Sqrt, bias=eps_ap)
    _recip_fast(nc, rstd, sdev[:])
    bc = psum_pool.tile([128, 2, tw], F32, tag=f"{name}_bc", bufs=bc_bufs,
                        name=f"{name}_bc")
    nc.tensor.matmul(bc[:, 0, :], ones_r, mu, start=True, stop=True)
    nc.tensor.matmul(bc[:, 1, :], ones_r, rstd, start=True, stop=True)
    for c in range(NCH):
        t1 = pool.tile([128, tw], F32, tag=f"{name}_t1", bufs=2,
                       name=f"{name}_t1")
        nc.vector.tensor_sub(t1[:], x_chunks(c), bc[:, 0, :])
        t2 = pool.tile([128, tw], F32, tag=f"{name}_t2", bufs=2,
                       name=f"{name}_t2")
        nc.vector.tensor_mul(t2[:], t1[:], bc[:, 1, :])
        out_write(c, t2[:], g_ap(c), b_ap(c))


def _build():
    nc = bacc.Bacc(None, target_bir_lowering=False)

    xT = nc.dram_tensor("xT", [D, T_EXT], BF16, kind="ExternalInput")
    wq = nc.dram_tensor("wq", [D, D], BF16, kind="ExternalInput")
    wk = nc.dram_tensor("wk", [D, D], BF16, kind="ExternalInput")
    wv = nc.dram_tensor("wv", [D, D], BF16, kind="ExternalInput")
    wo = nc.dram_tensor("wo", [D, D], BF16, kind="ExternalInput")
    w1r = nc.dram_tensor("w1r", [NFF, NCH, 128, 128], BF16, kind="ExternalInput")
    w2 = nc.dram_tensor("w2", [FF, D], BF16, kind="ExternalInput")
    b1c = nc.dram_tensor("b1c", [128, NFF], F32, kind="ExternalInput")
    b2c = nc.dram_tensor("b2c", [128, NCH], F32, kind="ExternalInput")
    g1c = nc.dram_tensor("g1c", [128, NCH], F32, kind="ExternalInput")
    bb1c = nc.dram_tensor("bb1c", [128, NCH], F32, kind="ExternalInput")
    g2c = nc.dram_tensor("g2c", [128, NCH], F32, kind="ExternalInput")
    bb2c = nc.dram_tensor("bb2c", [128, NCH], F32, kind="ExternalInput")
    amask = nc.dram_tensor("amask", [128, 1024], BF16, kind="ExternalInput")
    halo01 = nc.dram_tensor("halo01", [128, 1], F32, kind="ExternalInput")
    ones_cd = nc.dram_tensor("ones_cd", [128, 1], BF16, kind="ExternalInput")
    ones_rd = nc.dram_tensor("ones_rd", [1, 128], F32R, kind="ExternalInput")
    out = nc.dram_tensor("out", [D, T_OWN], F32, kind="ExternalOutput")

    with tile.TileContext(nc) as tc:
        const = tc.alloc_tile_pool(name="const", bufs=1)
        ones_c = const.tile([128, 1], BF16)
        nc.sync.dma_start(ones_c[:], ones_cd[:])
        ones_r = const.tile([1, 128], F32R)
        nc.sync.dma_start(ones_r[:], ones_rd[:])
        eps_sb = const.tile([1, 1], F32)
        nc.vector.memset(eps_sb[:], EPS)
        b1_sb = const.tile([128, NFF], F32)
        nc.sync.dma_start(b1_sb[:], b1c[:])
        b2_sb = const.tile([128, NCH], F32)
        nc.sync.dma_start(b2_sb[:], b2c[:])
        g1_sb = const.tile([128, NCH], F32)
        nc.sync.dma_start(g1_sb[:], g1c[:])
        bb1_sb = const.tile([128, NCH], F32)
        nc.sync.dma_start(bb1_sb[:], bb1c[:])
        g2_sb = const.tile([128, NCH], F32)
        nc.sync.dma_start(g2_sb[:], g2c[:])
        bb2_sb = const.tile([128, NCH], F32)
        nc.sync.dma_start(bb2_sb[:], bb2c[:])

        # persistent x1/x2 buffer (attn+residual output, then LN1 in place)
        x12_pool = tc.alloc_tile_pool(name="x12_pool", bufs=1)
        x12_sb = x12_pool.tile([128, NCH, T_OWN], BF16)

        # resident X_ext (feature-major, bf16)
        xe_pool = tc.alloc_tile_pool(name="xe_pool", bufs=1)
        xe_sb = xe_pool.tile([128, NCH, T_EXT], BF16)
        for c in range(NCH):
            nc.sync.dma_start(xe_sb[:, c, :], xT[c * 128:(c + 1) * 128, :])

        # ---------------- Stage A: QKV projections ----------------
        qkv = tc.alloc_tile_pool(name="qkv", bufs=1)
        q_sb = qkv.tile([128, NCH, T_OWN], BF16)     # own tokens only
        k_sb = qkv.tile([128, NCH, T_EXT], BF16)
        v_sb = qkv.tile([128, NTC, H * VW], BF16)

        wo_pool = tc.alloc_tile_pool(name="wo_pool", bufs=1)
        wo_sb = wo_pool.tile([128, NCH, D], BF16)
        for c in range(NCH):
            nc.sync.dma_start(wo_sb[:, c, :], wo[c * 128:(c + 1) * 128, :])

        pa = tc.alloc_tile_pool(name="pa", bufs=1)
        psa = tc.alloc_tile_pool(name="psa", bufs=1, space="PSUM")

        # q (own tokens), k (ext tokens): stream weight slices per dout tile
        for w_dram, dst, toff, tlen in ((wq, q_sb, BLEN, T_OWN),
                                        (wk, k_sb, 0, T_EXT)):
            for dt in range(NCH):
                w_t = pa.tile([128, NCH, 128], BF16, tag="w_t", bufs=2,
                              name="w_t")
                nc.sync.dma_start(
                    w_t[:],
                    w_dram[:, dt * 128:(dt + 1) * 128].rearrange(
                        "(c p) j -> p c j", p=128))
                ntile = (tlen + 511) // 512
                for ti in range(ntile):
                    t0 = ti * 512
                    tw = min(512, tlen - t0)
                    ps = psa.tile([128, 512], F32, tag="qk_ps", bufs=2,
                                  name="qk_ps")
                    for c in range(NCH):
                        nc.tensor.matmul(
                            ps[:, 0:tw], w_t[:, c, :],
                            xe_sb[:, c, toff + t0:toff + t0 + tw],
                            start=(c == 0), stop=(c == NCH - 1))
                    if dst is q_sb:
                        nc.scalar.copy(dst[:, dt, t0:t0 + tw], ps[:, 0:tw])
                    else:
                        nc.vector.tensor_copy(dst[:, dt, t0:t0 + tw],
                                              ps[:, 0:tw])
        # v token-major with 65-stride head layout (ones col last)
        for half in range(2):
            wv_t = pa.tile([128, NCH, 512], BF16, tag="wv_t", bufs=1,
                           name="wv_t")
            nc.sync.dma_start(
                wv_t[:],
                wv[:, half * 512:(half + 1) * 512].rearrange(
                    "(c p) j -> p c j", p=128))
            for tc_g in range(NTC):
                vv = v_sb[:, tc_g, :].rearrange("p (h e) -> p h e", e=VW)
                ps = psa.tile([128, 512], F32, tag="v_ps", bufs=2,
                              name="v_ps")
                for c in range(NCH):
                    nc.tensor.matmul(
                        ps[:], xe_sb[:, c, tc_g * 128:(tc_g + 1) * 128],
                        wv_t[:, c, :],
                        start=(c == 0), stop=(c == NCH - 1))
                nc.vector.tensor_copy(
                    vv[:, half * 8:(half + 1) * 8, 0:DH],
                    ps[:].rearrange("p (h e) -> p h e", e=DH))
                if half == 0:
                    nc.vector.memset(vv[:, :, DH:VW], 1.0)

        psa.release()
        pa.release()

        # ------- Stage B: attention + Wo + residual, fused per block -------
        pb = tc.alloc_tile_pool(name="pb", bufs=1)
        psb = tc.alloc_tile_pool(name="psb", bufs=1, space="PSUM")
        amask_sb = pb.tile([128, 4, 256], BF16, tag="amask", bufs=1,
                           name="amask_sb")
        nc.sync.dma_start(amask_sb[:],
                          amask[:].rearrange("p (k q) -> p k q", k=4))
        halo_sb = pb.tile([128, 1], F32, tag="halo", bufs=1, name="halo_sb")
        nc.sync.dma_start(halo_sb[:], halo01[:])

        for qb in range(QB):
            q0 = qb * BLEN                # own-token index of query block
            k0 = qb * BLEN                # ext-token index of first key
            o_t = pb.tile([128, NCH, BLEN], BF16, tag="o_t", bufs=1,
                          name="o_t")
            for hp in range(H // 2):      # head pairs share a feature chunk
                st = psb.tile([128, 2, 4, 256], F32, tag="st", bufs=1,
                              name="st")
                for kc in range(4):       # interleave: row-groups 0/64 overlap
                    for sub in range(2):
                        p0 = sub * 64
                        nc.tensor.matmul(
                            st[:, sub, kc, :],
                            k_sb[p0:p0 + 64, hp, k0 + kc * 128:k0 + (kc + 1) * 128],
                            q_sb[p0:p0 + 64, hp, q0:q0 + BLEN],
                            start=True, stop=True)
                es = pb.tile([128, 2, 4, 256], BF16, tag="es", bufs=2,
                             name="es")
                nc.scalar.activation(es[:], st[:], AF.Exp)
                for sub in range(2):
                    nc.vector.tensor_mul(es[:, sub, 2:4, :], es[:, sub, 2:4, :],
                                         amask_sb[:, 2:4, :])
                if qb == 0:
                    nc.vector.tensor_scalar_mul(es[:, :, 0:2, :],
                                                es[:, :, 0:2, :], halo_sb[:])
                for sub in range(2):
                    h = 2 * hp + sub
                    p0 = sub * 64
                    oa = psb.tile([VW, 256], F32, tag="oa", bufs=2, name="oa")
                    for kc in range(4):
                        nc.tensor.matmul(
                            oa[:],
                            v_sb[:, qb * 2 + kc, h * VW:(h + 1) * VW],
                            es[:, sub, kc, :],
                            start=(kc == 0), stop=(kc == 3))
                    den = pb.tile([1, 256], F32, tag="den", bufs=1,
                                  name="den")
                    if sub == 0:
                        nc.scalar.copy(den[:], oa[DH:VW, :])
                    else:
                        nc.vector.tensor_copy(den[:], oa[DH:VW, :])
                    rc = pb.tile([1, 256], F32R, tag="rc", bufs=1, name="rc")
                    _recip_fast(nc, rc[:], den[:])
                    bc = psb.tile([64, 256], F32, tag="att_bc", bufs=1,
                                  name="att_bc")
                    nc.tensor.matmul(bc[:], ones_r[0:1, 0:64], rc[:],
                                     start=True, stop=True)
                    ou = pb.tile([64, 256], BF16, tag="ou", bufs=2, name="ou")
                    if sub == 0:
                        nc.vector.tensor_copy(ou[:], oa[0:DH, :])
                    else:
                        nc.scalar.copy(ou[:], oa[0:DH, :])
                    nc.vector.tensor_mul(o_t[p0:p0 + 64, hp, :], ou[:], bc[:])
            # Wo projection + residual for this 256-token block
            for dt in range(NCH):
                y_ps = psb.tile([128, 256], F32, tag="y_ps", bufs=1,
                                name="y_ps")
                for c in range(NCH):
                    nc.tensor.matmul(y_ps[:],
                                     wo_sb[:, c, dt * 128:(dt + 1) * 128],
                                     o_t[:, c, :],
                                     start=(c == 0), stop=(c == NCH - 1))
                nc.vector.tensor_add(x12_sb[:, dt, q0:q0 + BLEN],
                                     xe_sb[:, dt, BLEN + q0:BLEN + q0 + BLEN],
                                     y_ps[:])

        psb.release()
        pb.release()
        wo_pool.release()
        qkv.release()
        xe_pool.release()

        # ------------ Stage C+D: LN1, then FFN + residual + LN2 ------------
        w2_pool = tc.alloc_tile_pool(name="w2_pool", bufs=1)
        w2_sb = w2_pool.tile([128, NFF, D], BF16)
        for f in range(NFF):
            nc.sync.dma_start(w2_sb[:, f, :], w2[f * 128:(f + 1) * 128, :])

        pd = tc.alloc_tile_pool(name="pd", bufs=1)
        psd = tc.alloc_tile_pool(name="psd", bufs=1, space="PSUM")

        for tt in range(4):
            t0 = tt * 512

            def wr1(c, t2ap, g_ap, b_ap, _t0=t0):
                nc.scalar.activation(x12_sb[:, c, _t0:_t0 + 512], t2ap,
                                     AF.Identity, bias=b_ap, scale=g_ap)

            _ln_fm(nc, pd, psd,
                   (lambda c, _t0=t0: x12_sb[:, c, _t0:_t0 + 512]), 512,
                   ones_c[:], ones_r[:], eps_sb[:],
                   (lambda c: g1_sb[:, c:c + 1]),
                   (lambda c: bb1_sb[:, c:c + 1]),
                   wr1, 1.0 / D, "ln", bc_bufs=1)

        for tt in range(4):
            t0 = tt * 512
            h_sb = pd.tile([128, NFF, 512], BF16, tag="h_sb", bufs=1,
                           name="h_sb")
            for f in range(NFF):
                w1_t = pd.tile([128, NCH, 128], BF16, tag="w1_t", bufs=4,
                               name="w1_t")
                nc.sync.dma_start(
                    w1_t[:], w1r[f, :, :, :].rearrange("c p j -> p c j"))
                h_ps = psd.tile([128, 512], F32, tag="h_ps", bufs=2,
                                name="h_ps")
                for c in range(NCH):
                    nc.tensor.matmul(
                        h_ps[:], w1_t[:, c, :],
                        x12_sb[:, c, t0:t0 + 512],
                        start=(c == 0), stop=(c == NCH - 1))
                nc.scalar.activation(h_sb[:, f, :], h_ps[:], AF.Relu,
                                     bias=b1_sb[:, f:f + 1])
            x3 = pd.tile([128, NCH, 512], BF16, tag="x3", bufs=2, name="x3")
            for dt in range(NCH):
                y2_ps = psd.tile([128, 512], F32, tag="y2_ps", bufs=2,
                                 name="y2_ps")
                for f in range(NFF):
                    nc.tensor.matmul(
                        y2_ps[:], w2_sb[:, f, dt * 128:(dt + 1) * 128],
                        h_sb[:, f, :],
                        start=(f == 0), stop=(f == NFF - 1))
                yb = pd.tile([128, 512], BF16, tag="yb", bufs=2, name="yb")
                nc.scalar.activation(yb[:], y2_ps[:], AF.Identity,
                                     bias=b2_sb[:, dt:dt + 1])
                nc.vector.tensor_add(x3[:, dt, :], yb[:],
                                     x12_sb[:, dt, t0:t0 + 512])

            def wr2(c, t2ap, g_ap, b_ap, _t0=t0):
                fin = pd.tile([128, 512], F32, tag="fin", bufs=3,
                              name="fin")
                nc.scalar.activation(fin[:], t2ap, AF.Identity,
                                     bias=b_ap, scale=g_ap)
                nc.sync.dma_start(
                    out[c * 128:(c + 1) * 128, _t0:_t0 + 512], fin[:])

            _ln_fm(nc, pd, psd,
                   (lambda c, _x3=x3: _x3[:, c, :]), 512, ones_c[:],
                   ones_r[:], eps_sb[:],
                   (lambda c: g2_sb[:, c:c + 1]),
                   (lambda c: bb2_sb[:, c:c + 1]),
                   wr2, 1.0 / D, "ln")

        psd.release()
        pd.release()
        w2_pool.release()
        x12_pool.release()
        const.release()

    nc.finalize()
    return nc


def _prep_inputs(X, Wq, Wk, Wv, Wo, g_attn, b_attn, W1, b1, W2, b2,
                 g_ffn, b_ffn):
    flat = np.ascontiguousarray(X.reshape(B * S, D))
    wq_b = (Wq * (DH ** -0.5)).astype(BFNP)
    wk_b = Wk.astype(BFNP)
    wv_b = Wv.astype(BFNP)
    wo_b = Wo.astype(BFNP)
    w1r = np.ascontiguousarray(
        W1.reshape(NCH, 128, NFF, 128).transpose(2, 0, 1, 3)).astype(BFNP)
    w2_b = W2.astype(BFNP)
    b1c = np.ascontiguousarray(b1.reshape(NFF, 128).T).astype(np.float32)
    b2c = np.ascontiguousarray(b2.reshape(NCH, 128).T).astype(np.float32)
    g1c = np.ascontiguousarray(g_attn.reshape(NCH, 128).T).astype(np.float32)
    bb1c = np.ascontiguousarray(b_attn.reshape(NCH, 128).T).astype(np.float32)
    g2c = np.ascontiguousarray(g_ffn.reshape(NCH, 128).T).astype(np.float32)
    bb2c = np.ascontiguousarray(b_ffn.reshape(NCH, 128).T).astype(np.float32)

    # multiplicative tail mask in S^T layout [key_row, (chunk, q_col)]:
    # chunk2 keeps keys r <= q i; chunk3 keeps i >= r + 128.
    r = np.arange(128)[:, None]
    i = np.arange(256)[None, :]
    tri2 = (r <= i).astype(np.float32)
    tri3 = (i >= r + 128).astype(np.float32)

    ones_cd = np.ones((128, 1), BFNP)
    ones_rd = np.ones((1, 128), np.float32)

    shared = dict(wq=wq_b, wk=wk_b, wv=wv_b, wo=wo_b, w1r=w1r, w2=w2_b,
                  b1c=b1c, b2c=b2c, g1c=g1c, bb1c=bb1c, g2c=g2c, bb2c=bb2c,
                  ones_cd=ones_cd, ones_rd=ones_rd)

    in_maps = []
    for core in range(NCORES):
        t0 = core * T_OWN
        ext = np.zeros((T_EXT, D), np.float32)
        ext[BLEN:] = flat[t0:t0 + T_OWN]
        starts_seq = (t0 % S) == 0
        if not starts_seq:
            ext[0:BLEN] = flat[t0 - BLEN:t0]
        xTc = np.ascontiguousarray(ext.T).astype(BFNP)
        am = np.empty((128, 1024), np.float32)
        am[:, 0:512] = 1.0
        am[:, 512:768] = tri2
        am[:, 768:1024] = tri3
        m = dict(shared)
        m["xT"] = xTc
        m["amask"] = am.astype(BFNP)
        m["halo01"] = np.full((128, 1), 0.0 if starts_seq else 1.0, np.float32)
        in_maps.append(m)
    return in_maps


def kernel(**inputs):
    if "nc" not in _CACHE:
        _CACHE["nc"] = _build()
    nc = _CACHE["nc"]
    in_maps = _prep_inputs(**inputs)
    trace = bool(int(os.environ.get("KERNEL_TRACE", "0")))
    if trace:
        sys.path.insert(0, os.path.dirname(os.path.abspath(__file__)))
        import types
        if "antenv.axon_hooks" not in sys.modules:
            import antenv
            from trn_agent_boot.trn_boot import _ntff_profile_via_ctypes
            hooks = types.ModuleType("antenv.axon_hooks")
            _hook = _ntff_profile_via_ctypes("/opt/axon/libaxon_pjrt.so")
            hooks.get_axon_ntff_profile_hook = lambda: _hook
            hooks.set_axon_ntff_profile_hook = lambda h: None
            sys.modules["antenv.axon_hooks"] = hooks
            antenv.axon_hooks = hooks
    res = run_bass_kernel_spmd(nc, in_maps, core_ids=list(range(NCORES)),
                               trace=trace)
    _CACHE["exec_time_ns"] = res.exec_time_ns
    out_flat = np.empty((B * S, D), np.float32)
    for core in range(NCORES):
        t0 = core * T_OWN
        out_flat[t0:t0 + T_OWN] = res.results[core]["out"].T
    return out_flat.reshape(B, S, D)


# revision 38
# speedup vs baseline: 1.1641x; 1.0302x over previous
"""Trainium2 Bass kernel for nn_DecoderLayer_54855322304772.

Decoder layer with block-local attention (BLEN=256, prev+current block),
B=4, S=4096, D=1024, H=16, FF=4096.

Strategy: data-parallel over 8 cores. Tokens flattened (B*S = 16384) and
split into 8 shards of 2048 tokens (8 blocks of 256). Each core gets a
256-token halo block before its shard (zeros + full mask when the shard
starts a new sequence). No collectives.

All activations are kept feature-major ([D, T]) in SBUF so every matmul
consumes them directly (TensorE contracts over the partition axis).
LayerNorm is computed feature-major: partition-dim sums via ones-vector
matmuls, per-token stats broadcast back across partitions via K=1 f32r
matmuls. Softmax skips max-subtraction (logits are O(1); masked logits
get -1e9 and underflow to exactly 0 in exp). The softmax denominator
comes for free from a ones-column appended to V.
"""

import os
import sys

sys.path.insert(0, "/opt/trn_rl_repo")

import numpy as np
import ml_dtypes

import concourse.bass as bass
import concourse.tile as tile
from concourse import bacc, mybir
from concourse import bass_utils as _bass_utils
from concourse.bass_utils import run_bass_kernel_spmd
from concourse.dve_ops import RECIPROCAL_APPROX_FAST, RECIP_APPROX_FAST_CONSTS


_RC = RECIP_APPROX_FAST_CONSTS


def _recip_fast(nc, out_ap, in_ap):
    """~18-bit 1/x in a single DVE op; out may be f32r (feeds matmuls)."""
    nc.vector._custom_dve(RECIPROCAL_APPROX_FAST, out=out_ap, in0=in_ap,
                          s0=_RC["s0"], s1=_RC["s1"], imm2=_RC["imm2"])

F32 = mybir.dt.float32
F32R = mybir.dt.float32r
BF16 = mybir.dt.bfloat16
AF = mybir.ActivationFunctionType
AO = mybir.AluOpType
BFNP = ml_dtypes.bfloat16

B, S, D, H, BLEN, FF = 4, 4096, 1024, 16, 256, 4096
DH = D // H          # 64
NEG = -1e9
EPS = 1e-6
NCORES = 8
T_OWN = (B * S) // NCORES          # 2048 tokens per core
T_EXT = T_OWN + BLEN               # 2304 with halo block
NCH = D // 128                     # 8 feature chunks
NFF = FF // 128                    # 32 ff chunks
NTC = T_EXT // 128                 # 18 token chunks (for V)
QB = T_OWN // BLEN                 # 8 query blocks of 256
VW = DH + 1                        # 65: v columns per head incl. ones col

_CACHE = {}


def _ln_fm(nc, pool, psum_pool, x_chunks, tw, ones_c, ones_r, eps_ap, g_ap,
           b_ap, out_write, inv_d, name, bc_bufs=1):
    """Feature-major layernorm over D = 128*NCH (partition x chunk axis).

    x_chunks: callable c -> AP [128, tw] (bf16) input chunk c.
    out_write: callable (c, ap_f32_or_cast_src) ... we instead return per
      chunk via ACT with scale/bias into caller-provided destination:
      out_write(c, t2_ap) must emit the final op.
    """
    sums = psum_pool.tile([1, 2, tw], F32, tag=f"{name}_sums", bufs=1,
                          name=f"{name}_sums")
    sq = None
    for c in range(NCH):
        sq = pool.tile([128, tw], BF16, tag=f"{name}_sq", bufs=2,
                       name=f"{name}_sq")
        nc.scalar.activation(sq[:], x_chunks(c), AF.Square)
        nc.tensor.matmul(sums[0:1, 1, :], ones_c, sq[:],
                         start=(c == 0), stop=(c == NCH - 1))
    for c in range(NCH):
        nc.tensor.matmul(sums[0:1, 0, :], ones_c, x_chunks(c),
                         start=(c == 0), stop=(c == NCH - 1))
    stats = pool.tile([1, 2, tw], F32R, tag=f"{name}_stats", bufs=1,
                      name=f"{name}_stats")
    mu = stats[0:1, 0, :]
    rstd = stats[0:1, 1, :]
    with nc.allow_low_precision(reason="f32r stats for broadcast matmul"):
        nc.scalar.mul(mu, sums[0:1, 0, :], inv_d)
    ex2 = pool.tile([1, tw], F32, tag=f"{name}_ex2", bufs=1, name=f"{name}_ex2")
    nc.scalar.mul(ex2[:], sums[0:1, 1, :], inv_d)
    mu2 = pool.tile([1, tw], F32, tag=f"{name}_mu2", bufs=1, name=f"{name}_mu2")
    nc.vector.tensor_mul(mu2[:], mu.bitcast(F32), mu.bitcast(F32))
    var = pool.tile([1, tw], F32, tag=f"{name}_var", bufs=1, name=f"{name}_var")
    nc.vector.tensor_sub(var[:], ex2[:], mu2[:])
    sdev = pool.tile([1, tw], F32, tag=f"{name}_sdev", bufs=1,
                     name=f"{name}_sdev")
    nc.scalar.activation(sdev[:], var[:], AF.Sqrt, bias=eps_ap)
    _recip_fast(nc, rstd, sdev[:])
    bc = psum_pool.tile([128, 2, tw], F32, tag=f"{name}_bc", bufs=bc_bufs,
                        name=f"{name}_bc")
    nc.tensor.matmul(bc[:, 0, :], ones_r, mu, start=True, stop=True)
    nc.tensor.matmul(bc[:, 1, :], ones_r, rstd, start=True, stop=True)
    for c in range(NCH):
        t1 = pool.tile([128, tw], F32, tag=f"{name}_t1", bufs=2,
                       name=f"{name}_t1")
        nc.vector.tensor_sub(t1[:], x_chunks(c), bc[:, 0, :])
        t2 = pool.tile([128, tw], F32, tag=f"{name}_t2", bufs=2,
                       name=f"{name}_t2")
        nc.vector.tensor_mul(t2[:], t1[:], bc[:, 1, :])
        out_write(c, t2[:], g_ap(c), b_ap(c))


def _build():
    nc = bacc.Bacc(None, target_bir_lowering=False)

    xT = nc.dram_tensor("xT", [D, T_EXT], BF16, kind="ExternalInput")
    wq = nc.dram_tensor("wq", [D, D], BF16, kind="ExternalInput")
    wk = nc.dram_tensor("wk", [D, D], BF16, kind="ExternalInput")
    wv = nc.dram_tensor("wv", [D, D], BF16, kind="ExternalInput")
    wo = nc.dram_tensor("wo", [D, D], BF16, kind="ExternalInput")
    w1r = nc.dram_tensor("w1r", [NFF, NCH, 128, 128], BF16, kind="ExternalInput")
    w2 = nc.dram_tensor("w2", [FF, D], BF16, kind="ExternalInput")
    b1c = nc.dram_tensor("b1c", [128, NFF], F32, kind="ExternalInput")
    b2c = nc.dram_tensor("b2c", [128, NCH], F32, kind="ExternalInput")
    g1c = nc.dram_tensor("g1c", [128, NCH], F32, kind="ExternalInput")
    bb1c = nc.dram_tensor("bb1c", [128, NCH], F32, kind="ExternalInput")
    g2c = nc.dram_tensor("g2c", [128, NCH], F32, kind="ExternalInput")
    bb2c = nc.dram_tensor("bb2c", [128, NCH], F32, kind="ExternalInput")
    amask = nc.dram_tensor("amask", [128, 512], BF16, kind="ExternalInput")
    halo01 = nc.dram_tensor("halo01", [128, 1], F32, kind="ExternalInput")
    ones_cd = nc.dram_tensor("ones_cd", [128, 1], BF16, kind="ExternalInput")
    ones_rd = nc.dram_tensor("ones_rd", [1, 128], F32R, kind="ExternalInput")
    out = nc.dram_tensor("out", [D, T_OWN], F32, kind="ExternalOutput")

    with tile.TileContext(nc) as tc:
        const = tc.alloc_tile_pool(name="const", bufs=1)
        ones_c = const.tile([128, 1], BF16)
        nc.sync.dma_start(ones_c[:], ones_cd[:])
        ones_r = const.tile([1, 128], F32R)
        nc.sync.dma_start(ones_r[:], ones_rd[:])
        eps_sb = const.tile([1, 1], F32)
        nc.vector.memset(eps_sb[:], EPS)
        b1_sb = const.tile([128, NFF], F32)
        nc.sync.dma_start(b1_sb[:], b1c[:])
        b2_sb = const.tile([128, NCH], F32)
        nc.sync.dma_start(b2_sb[:], b2c[:])
        g1_sb = const.tile([128, NCH], F32)
        nc.sync.dma_start(g1_sb[:], g1c[:])
        bb1_sb = const.tile([128, NCH], F32)
        nc.sync.dma_start(bb1_sb[:], bb1c[:])
        g2_sb = const.tile([128, NCH], F32)
        nc.sync.dma_start(g2_sb[:], g2c[:])
        bb2_sb = const.tile([128, NCH], F32)
        nc.sync.dma_start(bb2_sb[:], bb2c[:])

        # persistent x1/x2 buffer (attn+residual output, then LN1 in place)
        x12_pool = tc.alloc_tile_pool(name="x12_pool", bufs=1)
        x12_sb = x12_pool.tile([128, NCH, T_OWN], BF16)

        # resident X_ext (feature-major, bf16)
        xe_pool = tc.alloc_tile_pool(name="xe_pool", bufs=1)
        xeh_sb = xe_pool.tile([128, NCH, BLEN], BF16)
        xeo_sb = xe_pool.tile([128, NCH, T_OWN], BF16)
        for c in range(NCH):
            nc.sync.dma_start(xeh_sb[:, c, :], xT[c * 128:(c + 1) * 128, 0:BLEN])
            nc.sync.dma_start(xeo_sb[:, c, :], xT[c * 128:(c + 1) * 128, BLEN:])

        # ---------------- Stage A: QKV projections ----------------
        qkv = tc.alloc_tile_pool(name="qkv", bufs=1)
        q_sb = qkv.tile([128, NCH, T_OWN], BF16)     # own tokens only
        k_sb = qkv.tile([128, NCH, T_EXT], BF16)
        v_sb = qkv.tile([128, NTC, H * VW], BF16)

        wo_pool = tc.alloc_tile_pool(name="wo_pool", bufs=1)
        wo_sb = wo_pool.tile([128, NCH, D], BF16)
        for c in range(NCH):
            nc.sync.dma_start(wo_sb[:, c, :], wo[c * 128:(c + 1) * 128, :])

        pa = tc.alloc_tile_pool(name="pa", bufs=1)
        psa = tc.alloc_tile_pool(name="psa", bufs=1, space="PSUM")

        # q (own tokens), k (ext tokens): stream weight slices per dout tile
        q_tiles = [(xeo_sb, t, 512, t) for t in range(0, T_OWN, 512)]
        k_tiles = [(xeh_sb, 0, BLEN, 0)] + \
                  [(xeo_sb, t, 512, BLEN + t) for t in range(0, T_OWN, 512)]
        for w_dram, dst, tiles in ((wq, q_sb, q_tiles), (wk, k_sb, k_tiles)):
            for dt in range(NCH):
                w_t = pa.tile([128, NCH, 128], BF16, tag="w_t", bufs=2,
                              name="w_t")
                nc.sync.dma_start(
                    w_t[:],
                    w_dram[:, dt * 128:(dt + 1) * 128].rearrange(
                        "(c p) j -> p c j", p=128))
                for (src, s0, tw, d0) in tiles:
                    ps = psa.tile([128, 512], F32, tag="qk_ps", bufs=2,
                                  name="qk_ps")
                    for c in range(NCH):
                        nc.tensor.matmul(
                            ps[:, 0:tw], w_t[:, c, :],
                            src[:, c, s0:s0 + tw],
                            start=(c == 0), stop=(c == NCH - 1))
                    if dst is q_sb:
                        nc.scalar.copy(dst[:, dt, d0:d0 + tw], ps[:, 0:tw])
                    else:
                        nc.vector.tensor_copy(dst[:, dt, d0:d0 + tw],
                                              ps[:, 0:tw])
        # v token-major with 65-stride head layout (ones col last)
        for half in range(2):
            wv_t = pa.tile([128, NCH, 512], BF16, tag="wv_t", bufs=1,
                           name="wv_t")
            nc.sync.dma_start(
                wv_t[:],
                wv[:, half * 512:(half + 1) * 512].rearrange(
                    "(c p) j -> p c j", p=128))
            for tc_g in range(NTC):
                vv = v_sb[:, tc_g, :].rearrange("p (h e) -> p h e", e=VW)
                ps = psa.tile([128, 512], F32, tag="v_ps", bufs=2,
                              name="v_ps")
                if tc_g < 2:
                    vsrc, vs0 = xeh_sb, tc_g * 128
                else:
                    vsrc, vs0 = xeo_sb, (tc_g - 2) * 128
                for c in range(NCH):
                    nc.tensor.matmul(
                        ps[:], vsrc[:, c, vs0:vs0 + 128],
                        wv_t[:, c, :],
                        start=(c == 0), stop=(c == NCH - 1))
                nc.vector.tensor_copy(
                    vv[:, half * 8:(half + 1) * 8, 0:DH],
                    ps[:].rearrange("p (h e) -> p h e", e=DH))
                if half == 0:
                    nc.vector.memset(vv[:, :, DH:VW], 1.0)

        psa.release()
        pa.release()

        # ------- Stage B: attention + Wo + residual, fused per block -------
        pb = tc.alloc_tile_pool(name="pb", bufs=1)
        psb = tc.alloc_tile_pool(name="psb", bufs=1, space="PSUM")
        amask_sb = pb.tile([128, 2, 256], BF16, tag="amask", bufs=1,
                           name="amask_sb")
        nc.sync.dma_start(amask_sb[:],
                          amask[:].rearrange("p (k q) -> p k q", k=2))
        halo_sb = pb.tile([128, 1], F32, tag="halo", bufs=1, name="halo_sb")
        nc.sync.dma_start(halo_sb[:], halo01[:])

        for qb in range(QB):
            q0 = qb * BLEN                # own-token index of query block
            k0 = qb * BLEN                # ext-token index of first key
            o_t = pb.tile([128, NCH, BLEN], BF16, tag="o_t", bufs=1,
                          name="o_t")
            for hp in range(H // 2):      # head pairs share a feature chunk
                st = psb.tile([128, 2, 4, 256], F32, tag="st", bufs=1,
                              name="st")
                for kc in range(4):       # interleave: row-groups 0/64 overlap
                    for sub in range(2):
                        p0 = sub * 64
                        nc.tensor.matmul(
                            st[:, sub, kc, :],
                            k_sb[p0:p0 + 64, hp, k0 + kc * 128:k0 + (kc + 1) * 128],
                            q_sb[p0:p0 + 64, hp, q0:q0 + BLEN],
                            start=True, stop=True)
                es = pb.tile([128, 2, 4, 256], BF16, tag="es", bufs=2,
                             name="es")
                nc.scalar.activation(es[:], st[:], AF.Exp)
                for sub in range(2):
                    nc.vector.tensor_mul(es[:, sub, 2:4, :], es[:, sub, 2:4, :],
                                         amask_sb[:])
                if qb == 0:
                    nc.vector.tensor_scalar_mul(es[:, :, 0:2, :],
                                                es[:, :, 0:2, :], halo_sb[:])
                oa = psb.tile([VW, 2, 256], F32, tag="oa", bufs=2, name="oa")
                for sub in range(2):
                    h = 2 * hp + sub
                    for kc in range(4):
                        nc.tensor.matmul(
                            oa[:, sub, :],
                            v_sb[:, qb * 2 + kc, h * VW:(h + 1) * VW],
                            es[:, sub, kc, :],
                            start=(kc == 0), stop=(kc == 3))
                den = pb.tile([1, 2, 256], F32, tag="den", bufs=1, name="den")
                nc.vector.tensor_copy(den[:], oa[DH:VW, :, :])
                rc = pb.tile([1, 2, 256], F32R, tag="rc", bufs=1, name="rc")
                _recip_fast(nc, rc[:].rearrange("p a b -> p (a b)"),
                            den[:].rearrange("p a b -> p (a b)"))
                bc = psb.tile([64, 2, 256], F32, tag="att_bc", bufs=1,
                              name="att_bc")
                for sub in range(2):
                    nc.tensor.matmul(bc[:, sub, :], ones_r[0:1, 0:64],
                                     rc[0:1, sub, :], start=True, stop=True)
                ou = pb.tile([64, 2, 256], BF16, tag="ou", bufs=1, name="ou")
                nc.scalar.copy(ou[:], oa[0:DH, :, :])
                for sub in range(2):
                    nc.vector.tensor_mul(o_t[sub * 64:sub * 64 + 64, hp, :],
                                         ou[:, sub, :], bc[:, sub, :])
            # Wo projection + residual for this 256-token block
            for dt in range(NCH):
                y_ps = psb.tile([128, 256], F32, tag="y_ps", bufs=1,
                                name="y_ps")
                for c in range(NCH):
                    nc.tensor.matmul(y_ps[:],
                                     wo_sb[:, c, dt * 128:(dt + 1) * 128],
                                     o_t[:, c, :],
                                     start=(c == 0), stop=(c == NCH - 1))
                nc.vector.tensor_add(x12_sb[:, dt, q0:q0 + BLEN],
                                     xeo_sb[:, dt, q0:q0 + BLEN],
                                     y_ps[:])

        psb.release()
        pb.release()
        wo_pool.release()
        qkv.release()
        xe_pool.release()

        # ------------ Stage C+D: LN1, then FFN + residual + LN2 ------------
        w2_pool = tc.alloc_tile_pool(name="w2_pool", bufs=1)
        w2_sb = w2_pool.tile([128, NFF, D], BF16)
        for f in range(NFF):
            nc.sync.dma_start(w2_sb[:, f, :], w2[f * 128:(f + 1) * 128, :])

        pd = tc.alloc_tile_pool(name="pd", bufs=1)
        psd = tc.alloc_tile_pool(name="psd", bufs=1, space="PSUM")

        for tt in range(4):
            t0 = tt * 512

            def wr1(c, t2ap, g_ap, b_ap, _t0=t0):
                nc.scalar.activation(x12_sb[:, c, _t0:_t0 + 512], t2ap,
                                     AF.Identity, bias=b_ap, scale=g_ap)

            _ln_fm(nc, pd, psd,
                   (lambda c, _t0=t0: x12_sb[:, c, _t0:_t0 + 512]), 512,
                   ones_c[:], ones_r[:], eps_sb[:],
                   (lambda c: g1_sb[:, c:c + 1]),
                   (lambda c: bb1_sb[:, c:c + 1]),
                   wr1, 1.0 / D, "ln", bc_bufs=1)

        for tt in range(4):
            t0 = tt * 512
            h_sb = pd.tile([128, NFF, 512], BF16, tag="h_sb", bufs=1,
                           name="h_sb")
            for f in range(NFF):
                w1_t = pd.tile([128, NCH, 128], BF16, tag="w1_t", bufs=4,
                               name="w1_t")
                nc.sync.dma_start(
                    w1_t[:], w1r[f, :, :, :].rearrange("c p j -> p c j"))
                h_ps = psd.tile([128, 512], F32, tag="h_ps", bufs=2,
                                name="h_ps")
                for c in range(NCH):
                    nc.tensor.matmul(
                        h_ps[:], w1_t[:, c, :],
                        x12_sb[:, c, t0:t0 + 512],
                        start=(c == 0), stop=(c == NCH - 1))
                nc.scalar.activation(h_sb[:, f, :], h_ps[:], AF.Relu,
                                     bias=b1_sb[:, f:f + 1])
            x3 = pd.tile([128, NCH, 512], BF16, tag="x3", bufs=2, name="x3")
            for dt in range(NCH):
                y2_ps = psd.tile([128, 512], F32, tag="y2_ps", bufs=2,
                                 name="y2_ps")
                for f in range(NFF):
                    nc.tensor.matmul(
                        y2_ps[:], w2_sb[:, f, dt * 128:(dt + 1) * 128],
                        h_sb[:, f, :],
                        start=(f == 0), stop=(f == NFF - 1))
                yb = pd.tile([128, 512], BF16, tag="yb", bufs=2, name="yb")
                nc.scalar.activation(yb[:], y2_ps[:], AF.Identity,
                                     bias=b2_sb[:, dt:dt + 1])
                nc.vector.tensor_add(x3[:, dt, :], yb[:],
                                     x12_sb[:, dt, t0:t0 + 512])

            def wr2(c, t2ap, g_ap, b_ap, _t0=t0):
                fin = pd.tile([128, 512], F32, tag="fin", bufs=3,
                              name="fin")
                nc.scalar.activation(fin[:], t2ap, AF.Identity,
                                     bias=b_ap, scale=g_ap)
                nc.sync.dma_start(
                    out[c * 128:(c + 1) * 128, _t0:_t0 + 512], fin[:])

            _ln_fm(nc, pd, psd,
                   (lambda c, _x3=x3: _x3[:, c, :]), 512, ones_c[:],
                   ones_r[:], eps_sb[:],
                   (lambda c: g2_sb[:, c:c + 1]),
                   (lambda c: bb2_sb[:, c:c + 1]),
                   wr2, 1.0 / D, "ln")

        psd.release()
        pd.release()
        w2_pool.release()
        x12_pool.release()
        const.release()

    nc.finalize()
    return nc


def _prep_inputs(X, Wq, Wk, Wv, Wo, g_attn, b_attn, W1, b1, W2, b2,
                 g_ffn, b_ffn):
    flat = np.ascontiguousarray(X.reshape(B * S, D))
    wq_b = (Wq * (DH ** -0.5)).astype(BFNP)
    wk_b = Wk.astype(BFNP)
    wv_b = Wv.astype(BFNP)
    wo_b = Wo.astype(BFNP)
    w1r = np.ascontiguousarray(
        W1.reshape(NCH, 128, NFF, 128).transpose(2, 0, 1, 3)).astype(BFNP)
    w2_b = W2.astype(BFNP)
    b1c = np.ascontiguousarray(b1.reshape(NFF, 128).T).astype(np.float32)
    b2c = np.ascontiguousarray(b2.reshape(NCH, 128).T).astype(np.float32)
    g1c = np.ascontiguousarray(g_attn.reshape(NCH, 128).T).astype(np.float32)
    bb1c = np.ascontiguousarray(b_attn.reshape(NCH, 128).T).astype(np.float32)
    g2c = np.ascontiguousarray(g_ffn.reshape(NCH, 128).T).astype(np.float32)
    bb2c = np.ascontiguousarray(b_ffn.reshape(NCH, 128).T).astype(np.float32)

    # multiplicative tail mask in S^T layout [key_row, (chunk, q_col)]:
    # chunk2 keeps keys r <= q i; chunk3 keeps i >= r + 128.
    r = np.arange(128)[:, None]
    i = np.arange(256)[None, :]
    tri2 = (r <= i).astype(np.float32)
    tri3 = (i >= r + 128).astype(np.float32)

    ones_cd = np.ones((128, 1), BFNP)
    ones_rd = np.ones((1, 128), np.float32)

    shared = dict(wq=wq_b, wk=wk_b, wv=wv_b, wo=wo_b, w1r=w1r, w2=w2_b,
                  b1c=b1c, b2c=b2c, g1c=g1c, bb1c=bb1c, g2c=g2c, bb2c=bb2c,
                  ones_cd=ones_cd, ones_rd=ones_rd)

    in_maps = []
    for core in range(NCORES):
        t0 = core * T_OWN
        ext = np.zeros((T_EXT, D), np.float32)
        ext[BLEN:] = flat[t0:t0 + T_OWN]
        starts_seq = (t0 % S) == 0
        if not starts_seq:
            ext[0:BLEN] = flat[t0 - BLEN:t0]
        xTc = np.ascontiguousarray(ext.T).astype(BFNP)
        am = np.empty((128, 512), np.float32)
        am[:, 0:256] = tri2
        am[:, 256:512] = tri3
        m = dict(shared)
        m["xT"] = xTc
        m["amask"] = am.astype(BFNP)
        m["halo01"] = np.full((128, 1), 0.0 if starts_seq else 1.0, np.float32)
        in_maps.append(m)
    return in_maps


def kernel(**inputs):
    if "nc" not in _CACHE:
        _CACHE["nc"] = _build()
    nc = _CACHE["nc"]
    in_maps = _prep_inputs(**inputs)
    trace = bool(int(os.environ.get("KERNEL_TRACE", "0")))
    if trace:
        sys.path.insert(0, os.path.dirname(os.path.abspath(__file__)))
        import types
        if "antenv.axon_hooks" not in sys.modules:
            import antenv
            from trn_agent_boot.trn_boot import _ntff_profile_via_ctypes
            hooks = types.ModuleType("antenv.axon_hooks")
            _hook = _ntff_profile_via_ctypes("/opt/axon/libaxon_pjrt.so")
            hooks.get_axon_ntff_profile_hook = lambda: _hook
            hooks.set_axon_ntff_profile_hook = lambda h: None
            sys.modules["antenv.axon_hooks"] = hooks
            antenv.axon_hooks = hooks
    res = run_bass_kernel_spmd(nc, in_maps, core_ids=list(range(NCORES)),
                               trace=trace)
    _CACHE["exec_time_ns"] = res.exec_time_ns
    out_flat = np.empty((B * S, D), np.float32)
    for core in range(NCORES):
        t0 = core * T_OWN
        out_flat[t0:t0 + T_OWN] = res.results[core]["out"].T
    return out_flat.reshape(B, S, D)
